# revision 5
# baseline (speedup 1.0000x reference)
"""Trainium2 kernel for nn_AdaFastFoodMergedModel.

FastFood transform: y = SCALE * Sel . H . diag(G) . Pi . H . diag(B) . x
(H = 4096-point orthonormal Walsh-Hadamard, Pi = random permutation,
Sel = row subset of size 1228).

Strategy: everything right of `x` is a fixed linear operator built from the
small inputs (B, G, Pi, row_idx), so fold it on the host into one dense
matrix W [4096, 1228] (bf16) and run y = x @ W on the TensorEngine.
Device work per core (rows sharded 8192/8 = 1024 rows):
  - DMA x row-tile [128, 4096] f32, cast to bf16 (DVE)
  - PE-transpose 128x128 chunks to get xT (feat-on-partitions) for lhsT
  - 32 K-chunk matmuls accumulate psum[rows=128, sel<=512] over feat
  - evacuate psum -> SBUF (ACT), DMA out f32
No cross-core communication (data parallel over rows).
"""

import math
import sys

import numpy as np

sys.path.insert(0, "/opt/trn_rl_repo")

import ml_dtypes

ROWS, D = 8192, 4096
M = 1228
SCALE = math.sqrt(D / M)
N_CORES = 8
SHARD = ROWS // N_CORES  # 1024
P = 128
KC = D // P  # 32 contraction chunks
RT = SHARD // P  # 8 row tiles per core
SEL_CHUNKS = [(0, 512), (512, 512), (1024, 204)]  # 1228 = 512+512+204

# set by test harness to collect a profile
TRACE = False
LAST = {}

_CACHE = {}


def _fwht_cols(a: np.ndarray) -> np.ndarray:
    """Orthonormal FWHT along axis 0 (Sylvester/natural order)."""
    n = a.shape[0]
    x = a.copy()
    h = 1
    while h < n:
        x = x.reshape(n // (2 * h), 2, h, -1)
        lo = x[:, 0]
        hi = x[:, 1]
        x = np.stack((lo + hi, lo - hi), axis=1).reshape(n, -1)
        h *= 2
    return x * (1.0 / math.sqrt(n))


def _build_w(B, G, Pi, row_idx) -> np.ndarray:
    """W such that y = x @ W  (float32)."""
    S = np.zeros((D, M), dtype=np.float64)
    S[row_idx, np.arange(M)] = 1.0  # Sel^T
    A = _fwht_cols(S)  # H .
    A = A * G[:, None].astype(np.float64)  # diag(G) .
    A2 = np.empty_like(A)
    A2[Pi] = A  # Pi^T .
    A2 = _fwht_cols(A2)  # H .
    W = SCALE * (B[:, None].astype(np.float64) * A2)  # diag(B) .
    return W.astype(np.float32)


def _install_ntff_shim():
    """The image's antenv lacks axon_hooks; provide it so
    run_bass_kernel_spmd(trace=True) can collect an NTFF profile."""
    import types

    try:
        import antenv.axon_hooks  # noqa: F401

        return
    except ImportError:
        pass
    try:
        from trn_agent_boot.trn_boot import _ntff_profile_via_ctypes

        hook = _ntff_profile_via_ctypes("/opt/axon/libaxon_pjrt.so")
    except Exception:
        hook = None
    mod = types.ModuleType("antenv.axon_hooks")
    mod.get_axon_ntff_profile_hook = lambda: hook
    mod.set_axon_ntff_profile_hook = lambda h: None
    sys.modules["antenv.axon_hooks"] = mod


def _build_bass():
    import concourse.bass as bass
    import concourse.bacc as bacc
    import concourse.mybir as mybir
    from concourse import tile

    f32 = mybir.dt.float32
    bf16 = mybir.dt.bfloat16

    nc = bacc.Bacc("TRN2", target_bir_lowering=False, debug=False)
    x_in = nc.declare_dram_parameter("x", [SHARD, D], f32, isOutput=False)
    w_in = nc.declare_dram_parameter("w", [D, M], bf16, isOutput=False)
    id_in = nc.declare_dram_parameter("ident", [P, P], bf16, isOutput=False)
    out = nc.declare_dram_parameter("out", [SHARD, M], f32, isOutput=True)

    with tile.TileContext(nc) as tc:
        with (
            tc.tile_pool(name="const", bufs=1) as const_pool,
            tc.tile_pool(name="xin", bufs=2) as x_pool,
            tc.tile_pool(name="xbf", bufs=2) as xbf_pool,
            tc.tile_pool(name="xT", bufs=2) as xT_pool,
            tc.tile_pool(name="y", bufs=2) as y_pool,
            tc.tile_pool(name="pst", bufs=2, space=bass.MemorySpace.PSUM) as pst_pool,
            tc.tile_pool(name="psy", bufs=2, space=bass.MemorySpace.PSUM) as psy_pool,
        ):
            w_sb = const_pool.tile([P, KC, M], bf16)
            nc.sync.dma_start(w_sb[:], w_in.rearrange("(kc p) m -> p kc m", p=P))
            ident = const_pool.tile([P, P], bf16)
            nc.sync.dma_start(ident[:], id_in[:])

            for rt in range(RT):
                xt32 = x_pool.tile([P, D], f32)
                nc.sync.dma_start(xt32[:], x_in[rt * P : (rt + 1) * P, :])
                xtb = xbf_pool.tile([P, D], bf16)
                nc.vector.tensor_copy(xtb[:], xt32[:])

                xT = xT_pool.tile([P, KC, P], bf16)
                for kc in range(KC):
                    pst = pst_pool.tile([P, P], bf16)
                    nc.tensor.transpose(
                        pst[:], xtb[:, kc * P : (kc + 1) * P], ident[:]
                    )
                    nc.vector.tensor_copy(xT[:, kc, :], pst[:])

                y_sb = y_pool.tile([P, M], f32)
                for off, sz in SEL_CHUNKS:
                    psy = psy_pool.tile([P, sz], f32, tag="psy")
                    for kc in range(KC):
                        nc.tensor.matmul(
                            psy[:],
                            xT[:, kc, :],
                            w_sb[:, kc, off : off + sz],
                            start=(kc == 0),
                            stop=(kc == KC - 1),
                        )
                    nc.scalar.copy(y_sb[:, off : off + sz], psy[:])
                nc.sync.dma_start(out[rt * P : (rt + 1) * P, :], y_sb[:])

    nc.compile()
    return nc


def kernel(x, B, G, Pi, row_idx):
    x = np.ascontiguousarray(np.asarray(x, dtype=np.float32))
    B = np.asarray(B, dtype=np.float32)
    G = np.asarray(G, dtype=np.float32)
    Pi = np.asarray(Pi, dtype=np.int32)
    row_idx = np.asarray(row_idx, dtype=np.int32)

    W = _build_w(B, G, Pi, row_idx).astype(ml_dtypes.bfloat16)
    ident = np.eye(P, dtype=ml_dtypes.bfloat16)

    if "nc" not in _CACHE:
        _CACHE["nc"] = _build_bass()
    nc = _CACHE["nc"]

    if TRACE:
        _install_ntff_shim()

    from concourse.bass_utils import run_bass_kernel_spmd

    shards = [x[i * SHARD : (i + 1) * SHARD] for i in range(N_CORES)]
    in_maps = [{"x": shards[i], "w": W, "ident": ident} for i in range(N_CORES)]

    res = run_bass_kernel_spmd(
        nc, in_maps, core_ids=list(range(N_CORES)), trace=TRACE
    )
    LAST["exec_time_ns"] = getattr(res, "exec_time_ns", None)
    LAST["results"] = res

    outs = [res.results[i]["out"] for i in range(N_CORES)]
    return np.concatenate(outs, axis=0).astype(np.float32)


if __name__ == "__main__":
    rng = np.random.default_rng(0)
    x = rng.standard_normal((ROWS, D), dtype=np.float32)
    B = (rng.integers(0, 2, D) * 2 - 1).astype(np.float32)
    G = rng.standard_normal(D, dtype=np.float32)
    Pi = rng.permutation(D).astype(np.int32)
    row_idx = rng.permutation(D)[:M].astype(np.int32)
    y = kernel(x=x, B=B, G=G, Pi=Pi, row_idx=row_idx)
    print("out", y.shape, y.dtype)


# revision 6
# speedup vs baseline: 1.0028x; 1.0028x over previous
"""Trainium2 kernel for nn_AdaFastFoodMergedModel.

FastFood transform: y = SCALE * Sel . H . diag(G) . Pi . H . diag(B) . x
(H = 4096-point orthonormal Walsh-Hadamard, Pi = random permutation,
Sel = row subset of size 1228).

Strategy: everything right of `x` is a fixed linear operator built from the
small inputs (B, G, Pi, row_idx), so fold it on the host into one dense
matrix W [4096, 1228] (bf16) and run y = x @ W on the TensorEngine.
Device work per core (rows sharded 8192/8 = 1024 rows):
  - DMA x row-tile [128, 4096] f32, cast to bf16 (DVE)
  - PE-transpose 128x128 chunks to get xT (feat-on-partitions) for lhsT
  - 32 K-chunk matmuls accumulate psum[rows=128, sel<=512] over feat
  - evacuate psum -> SBUF (ACT), DMA out f32
No cross-core communication (data parallel over rows).
"""

import math
import sys

import numpy as np

sys.path.insert(0, "/opt/trn_rl_repo")

import ml_dtypes

ROWS, D = 8192, 4096
M = 1228
SCALE = math.sqrt(D / M)
N_CORES = 8
SHARD = ROWS // N_CORES  # 1024
P = 128
KC = D // P  # 32 contraction chunks
RT = SHARD // P  # 8 row tiles per core
SEL_CHUNKS = [(0, 512), (512, 512), (1024, 204)]  # 1228 = 512+512+204

# set by test harness to collect a profile
TRACE = False
LAST = {}

_CACHE = {}


def _fwht_cols(a: np.ndarray) -> np.ndarray:
    """Orthonormal FWHT along axis 0 (Sylvester/natural order)."""
    n = a.shape[0]
    x = a.copy()
    h = 1
    while h < n:
        x = x.reshape(n // (2 * h), 2, h, -1)
        lo = x[:, 0]
        hi = x[:, 1]
        x = np.stack((lo + hi, lo - hi), axis=1).reshape(n, -1)
        h *= 2
    return x * (1.0 / math.sqrt(n))


def _build_w(B, G, Pi, row_idx) -> np.ndarray:
    """W such that y = x @ W  (float32)."""
    S = np.zeros((D, M), dtype=np.float64)
    S[row_idx, np.arange(M)] = 1.0  # Sel^T
    A = _fwht_cols(S)  # H .
    A = A * G[:, None].astype(np.float64)  # diag(G) .
    A2 = np.empty_like(A)
    A2[Pi] = A  # Pi^T .
    A2 = _fwht_cols(A2)  # H .
    W = SCALE * (B[:, None].astype(np.float64) * A2)  # diag(B) .
    return W.astype(np.float32)


def _install_ntff_shim():
    """The image's antenv lacks axon_hooks; provide it so
    run_bass_kernel_spmd(trace=True) can collect an NTFF profile."""
    import types

    try:
        import antenv.axon_hooks  # noqa: F401

        return
    except ImportError:
        pass
    try:
        from trn_agent_boot.trn_boot import _ntff_profile_via_ctypes

        hook = _ntff_profile_via_ctypes("/opt/axon/libaxon_pjrt.so")
    except Exception:
        hook = None
    mod = types.ModuleType("antenv.axon_hooks")
    mod.get_axon_ntff_profile_hook = lambda: hook
    mod.set_axon_ntff_profile_hook = lambda h: None
    sys.modules["antenv.axon_hooks"] = mod


def _build_bass():
    import concourse.bass as bass
    import concourse.bacc as bacc
    import concourse.mybir as mybir
    from concourse import tile

    f32 = mybir.dt.float32
    bf16 = mybir.dt.bfloat16

    nc = bacc.Bacc("TRN2", target_bir_lowering=False, debug=False)
    x_in = nc.declare_dram_parameter("x", [SHARD, D], f32, isOutput=False)
    w_in = nc.declare_dram_parameter("w", [D, M], bf16, isOutput=False)
    id_in = nc.declare_dram_parameter("ident", [P, P], bf16, isOutput=False)
    out = nc.declare_dram_parameter("out", [SHARD, M], f32, isOutput=True)

    with tile.TileContext(nc) as tc:
        with (
            tc.tile_pool(name="const", bufs=1) as const_pool,
            tc.tile_pool(name="xbf", bufs=3) as xbf_pool,
            tc.tile_pool(name="xT", bufs=2) as xT_pool,
            tc.tile_pool(name="y", bufs=2) as y_pool,
            tc.tile_pool(name="psy", bufs=3, space=bass.MemorySpace.PSUM) as psy_pool,
        ):
            w_view = w_in.rearrange("(kc p) m -> p kc m", p=P)
            w_sb = const_pool.tile([P, KC, M], bf16)
            for kc in range(KC):
                nc.sync.dma_start(w_sb[:, kc, :], w_view[:, kc, :])

            for rt in range(RT):
                # cast f32 -> bf16 in the DMA (SWDGE)
                xtb = xbf_pool.tile([P, D], bf16)
                nc.gpsimd.dma_start(xtb[:], x_in[rt * P : (rt + 1) * P, :])

                # whole-tile block transpose through the DMA xbar:
                # xT[p, kc, j] = xtb[j, kc*128+p]
                xT = xT_pool.tile([P, KC, P], bf16)
                nc.sync.dma_start(xT[:], xtb[:], transpose=True)

                y_sb = y_pool.tile([P, M], f32)
                for off, sz in SEL_CHUNKS:
                    psy = psy_pool.tile([P, sz], f32, tag="psy")
                    for kc in range(KC):
                        nc.tensor.matmul(
                            psy[:],
                            xT[:, kc, :],
                            w_sb[:, kc, off : off + sz],
                            start=(kc == 0),
                            stop=(kc == KC - 1),
                        )
                    nc.scalar.copy(y_sb[:, off : off + sz], psy[:])
                nc.sync.dma_start(out[rt * P : (rt + 1) * P, :], y_sb[:])

    nc.compile()
    return nc


def kernel(x, B, G, Pi, row_idx):
    x = np.ascontiguousarray(np.asarray(x, dtype=np.float32))
    B = np.asarray(B, dtype=np.float32)
    G = np.asarray(G, dtype=np.float32)
    Pi = np.asarray(Pi, dtype=np.int32)
    row_idx = np.asarray(row_idx, dtype=np.int32)

    W = _build_w(B, G, Pi, row_idx).astype(ml_dtypes.bfloat16)
    ident = np.eye(P, dtype=ml_dtypes.bfloat16)

    if "nc" not in _CACHE:
        _CACHE["nc"] = _build_bass()
    nc = _CACHE["nc"]

    if TRACE:
        _install_ntff_shim()

    from concourse.bass_utils import run_bass_kernel_spmd

    shards = [x[i * SHARD : (i + 1) * SHARD] for i in range(N_CORES)]
    in_maps = [{"x": shards[i], "w": W, "ident": ident} for i in range(N_CORES)]

    res = run_bass_kernel_spmd(
        nc, in_maps, core_ids=list(range(N_CORES)), trace=TRACE
    )
    LAST["exec_time_ns"] = getattr(res, "exec_time_ns", None)
    LAST["results"] = res

    outs = [res.results[i]["out"] for i in range(N_CORES)]
    return np.concatenate(outs, axis=0).astype(np.float32)


if __name__ == "__main__":
    rng = np.random.default_rng(0)
    x = rng.standard_normal((ROWS, D), dtype=np.float32)
    B = (rng.integers(0, 2, D) * 2 - 1).astype(np.float32)
    G = rng.standard_normal(D, dtype=np.float32)
    Pi = rng.permutation(D).astype(np.int32)
    row_idx = rng.permutation(D)[:M].astype(np.int32)
    y = kernel(x=x, B=B, G=G, Pi=Pi, row_idx=row_idx)
    print("out", y.shape, y.dtype)


# revision 7
# speedup vs baseline: 1.1493x; 1.1461x over previous
"""Trainium2 kernel for nn_AdaFastFoodMergedModel.

FastFood transform: y = SCALE * Sel . H . diag(G) . Pi . H . diag(B) . x
(H = 4096-point orthonormal Walsh-Hadamard, Pi = random permutation,
Sel = row subset of size 1228).

Strategy: everything right of `x` is a fixed linear operator built from the
small inputs (B, G, Pi, row_idx), so fold it on the host into one dense
matrix W [4096, 1228] (bf16) and run y = x @ W on the TensorEngine.
Device work per core (rows sharded 8192/8 = 1024 rows):
  - DMA x row-tile [128, 4096] f32, cast to bf16 (DVE)
  - PE-transpose 128x128 chunks to get xT (feat-on-partitions) for lhsT
  - 32 K-chunk matmuls accumulate psum[rows=128, sel<=512] over feat
  - evacuate psum -> SBUF (ACT), DMA out f32
No cross-core communication (data parallel over rows).
"""

import math
import sys

import numpy as np

sys.path.insert(0, "/opt/trn_rl_repo")

import ml_dtypes

ROWS, D = 8192, 4096
M = 1228
SCALE = math.sqrt(D / M)
N_CORES = 8
SHARD = ROWS // N_CORES  # 1024
P = 128
KC = D // P  # 32 contraction chunks
RT = SHARD // P  # 8 row tiles per core
SEL_CHUNKS = [(0, 512), (512, 512), (1024, 204)]  # 1228 = 512+512+204

# set by test harness to collect a profile
TRACE = False
LAST = {}

_CACHE = {}


def _fwht_cols(a: np.ndarray) -> np.ndarray:
    """Orthonormal FWHT along axis 0 (Sylvester/natural order)."""
    n = a.shape[0]
    x = a.copy()
    h = 1
    while h < n:
        x = x.reshape(n // (2 * h), 2, h, -1)
        lo = x[:, 0]
        hi = x[:, 1]
        x = np.stack((lo + hi, lo - hi), axis=1).reshape(n, -1)
        h *= 2
    return x * (1.0 / math.sqrt(n))


def _build_w(B, G, Pi, row_idx) -> np.ndarray:
    """W such that y = x @ W  (float32)."""
    S = np.zeros((D, M), dtype=np.float64)
    S[row_idx, np.arange(M)] = 1.0  # Sel^T
    A = _fwht_cols(S)  # H .
    A = A * G[:, None].astype(np.float64)  # diag(G) .
    A2 = np.empty_like(A)
    A2[Pi] = A  # Pi^T .
    A2 = _fwht_cols(A2)  # H .
    W = SCALE * (B[:, None].astype(np.float64) * A2)  # diag(B) .
    return W.astype(np.float32)


def _install_ntff_shim():
    """The image's antenv lacks axon_hooks; provide it so
    run_bass_kernel_spmd(trace=True) can collect an NTFF profile."""
    import types

    try:
        import antenv.axon_hooks  # noqa: F401

        return
    except ImportError:
        pass
    try:
        from trn_agent_boot.trn_boot import _ntff_profile_via_ctypes

        hook = _ntff_profile_via_ctypes("/opt/axon/libaxon_pjrt.so")
    except Exception:
        hook = None
    mod = types.ModuleType("antenv.axon_hooks")
    mod.get_axon_ntff_profile_hook = lambda: hook
    mod.set_axon_ntff_profile_hook = lambda h: None
    sys.modules["antenv.axon_hooks"] = mod


def _build_bass():
    import concourse.bass as bass
    import concourse.bacc as bacc
    import concourse.mybir as mybir
    from concourse import tile

    f32 = mybir.dt.float32
    bf16 = mybir.dt.bfloat16

    nc = bacc.Bacc("TRN2", target_bir_lowering=False, debug=False)
    x_in = nc.declare_dram_parameter("x", [SHARD, D], f32, isOutput=False)
    w_in = nc.declare_dram_parameter("w", [D, M], bf16, isOutput=False)
    id_in = nc.declare_dram_parameter("ident", [P, P], bf16, isOutput=False)
    out = nc.declare_dram_parameter("out", [SHARD, M], f32, isOutput=True)

    with tile.TileContext(nc) as tc:
        with (
            tc.tile_pool(name="const", bufs=1) as const_pool,
            tc.tile_pool(name="xbf", bufs=3) as xbf_pool,
            tc.tile_pool(name="xT", bufs=2) as xT_pool,
            tc.tile_pool(name="y", bufs=2) as y_pool,
            tc.tile_pool(name="psy", bufs=3, space=bass.MemorySpace.PSUM) as psy_pool,
        ):
            # first x tile in flight immediately (SWDGE ring, casts f32->bf16)
            xtb0 = xbf_pool.tile([P, D], bf16, tag="xtb")
            nc.gpsimd.dma_start(xtb0[:], x_in[0:P, :])

            # W chunks on the scalar-engine HWDGE ring, parallel to sync's
            w_view = w_in.rearrange("(kc p) m -> p kc m", p=P)
            w_sb = const_pool.tile([P, KC, M], bf16)
            for kc in range(KC):
                nc.scalar.dma_start(w_sb[:, kc, :], w_view[:, kc, :])

            for rt in range(RT):
                if rt == 0:
                    xtb = xtb0
                else:
                    # cast f32 -> bf16 in the DMA (SWDGE)
                    xtb = xbf_pool.tile([P, D], bf16, tag="xtb")
                    nc.gpsimd.dma_start(xtb[:], x_in[rt * P : (rt + 1) * P, :])

                # whole-tile block transpose through the DMA xbar:
                # xT[p, kc, j] = xtb[j, kc*128+p]
                xT = xT_pool.tile([P, KC, P], bf16)
                nc.sync.dma_start(xT[:], xtb[:], transpose=True)

                y_sb = y_pool.tile([P, M], f32)
                for off, sz in SEL_CHUNKS:
                    psy = psy_pool.tile([P, sz], f32, tag="psy")
                    for kc in range(KC):
                        nc.tensor.matmul(
                            psy[:],
                            xT[:, kc, :],
                            w_sb[:, kc, off : off + sz],
                            start=(kc == 0),
                            stop=(kc == KC - 1),
                        )
                    nc.scalar.copy(y_sb[:, off : off + sz], psy[:])
                nc.sync.dma_start(out[rt * P : (rt + 1) * P, :], y_sb[:])

    nc.compile()
    return nc


def kernel(x, B, G, Pi, row_idx):
    x = np.ascontiguousarray(np.asarray(x, dtype=np.float32))
    B = np.asarray(B, dtype=np.float32)
    G = np.asarray(G, dtype=np.float32)
    Pi = np.asarray(Pi, dtype=np.int32)
    row_idx = np.asarray(row_idx, dtype=np.int32)

    W = _build_w(B, G, Pi, row_idx).astype(ml_dtypes.bfloat16)
    ident = np.eye(P, dtype=ml_dtypes.bfloat16)

    if "nc" not in _CACHE:
        _CACHE["nc"] = _build_bass()
    nc = _CACHE["nc"]

    if TRACE:
        _install_ntff_shim()

    from concourse.bass_utils import run_bass_kernel_spmd

    shards = [x[i * SHARD : (i + 1) * SHARD] for i in range(N_CORES)]
    in_maps = [{"x": shards[i], "w": W, "ident": ident} for i in range(N_CORES)]

    res = run_bass_kernel_spmd(
        nc, in_maps, core_ids=list(range(N_CORES)), trace=TRACE
    )
    LAST["exec_time_ns"] = getattr(res, "exec_time_ns", None)
    LAST["results"] = res

    outs = [res.results[i]["out"] for i in range(N_CORES)]
    return np.concatenate(outs, axis=0).astype(np.float32)


if __name__ == "__main__":
    rng = np.random.default_rng(0)
    x = rng.standard_normal((ROWS, D), dtype=np.float32)
    B = (rng.integers(0, 2, D) * 2 - 1).astype(np.float32)
    G = rng.standard_normal(D, dtype=np.float32)
    Pi = rng.permutation(D).astype(np.int32)
    row_idx = rng.permutation(D)[:M].astype(np.int32)
    y = kernel(x=x, B=B, G=G, Pi=Pi, row_idx=row_idx)
    print("out", y.shape, y.dtype)
